# revision 4
# baseline (speedup 1.0000x reference)
"""Trainium2 Bass kernel for the DSVF (digital state-variable filter) scan.

The reference is a lax.scan of a 2-state linear time-invariant recurrence over
T=8192 steps, batched over B=2048 independent rows:

    v_{t+1} = A v_t + b x_t          (A: 2x2, b: 2-vector, batch-independent)
    y_t     = c . v_t + d x_t

For the filter parameters in play the eigenvalues of A satisfy |lambda| < 1,
so the impulse response h_m (h_0 = d, h_m = c A^{m-1} b) decays geometrically;
with |lambda| ~= 0.655 the taps beyond ~64 are < 1e-12 in float64.  The scan is
therefore exactly (to fp32 precision) a causal FIR filter of <=128 taps plus a
rank-2 initial-state correction, and v_final is a 128-tap windowed sum of the
last input block:

    y[b, t]    = sum_{m=0..127} h_m x[b, t-m]  +  (c A^t) . v0[b]
    vfin[b, :] = sum_{j=0..127} (A^j b) x[b, T-1-j]        (A^T v0 underflows)

On-device layout (pure data parallel over 8 cores, 256 batch rows each):
  - x arrives [256, 8192] with batch on SBUF partitions;
  - TensorE transposes 128x128 blocks to put time on partitions (4 blocks
    batched per PSUM bank so the PSUM->SBUF copy amortizes its fixed cost);
  - the FIR becomes banded-Toeplitz matmuls: per output chunk, accumulating
    matmuls against overlapping slices of one [128, 512] band matrix
    (lookback block, current block(s));
  - output chunks accumulate in a 2-bank PSUM tile, copied to SBUF in
    [128, 1024] pieces (DVE/ACT alternating), then DMA'd out.
"""

import sys

sys.path.insert(0, "/opt/trn_rl_repo")

import numpy as np

import concourse.mybir as mybir
import concourse.tile as tile
from concourse import bacc, masks
from concourse.bass_utils import run_bass_kernel_spmd

F32 = mybir.dt.float32
F32R = mybir.dt.float32r

N_CORES = 8
B, T = 2048, 8192
B_CORE = B // N_CORES  # 256 batch rows per core
NGROUP = B_CORE // 128  # 2 partition groups of 128 rows
NBLK = T // 128  # 64 time blocks of 128
QCOL = 2048  # x input DMA width (1 MiB per DMA)
OSTAGE = 1024  # y output staging width (512 KiB per DMA)
XTW = 512  # transposed-block staging width (4 blocks / PSUM bank)

# Band support: one 128-block of lookback is wired into the matmul structure,
# so taps up to m=128 are represented exactly.  h beyond that must be
# negligible (checked at table-build time).
HLEN = 384  # h_m evaluated for m in [0, HLEN); zero-padded into the band


def _filter_tables(G, twoR, hp_gain, bp_gain, lp_gain, master_gain, chunk):
    """Host-side float64 precompute of the FIR band matrix and corrections."""
    G = float(np.clip(G, 1e-8, None))
    twoR = float(np.clip(twoR, 0.0, None))
    bg = float(np.clip(bp_gain, -1.0, None))
    hg = float(np.clip(hp_gain, -1.0, 1.0))
    lg = float(np.clip(lp_gain, -1.0, 1.0))
    mg = float(master_gain)

    c0 = 1.0 / (1.0 + G * (G + twoR))
    c1 = G * c0
    A = np.array([[2 * c0 - 1, -2 * c1], [2 * G * c0, 1 - 2 * G * c1]])
    bvec = np.array([2 * c1, 2 * G * c1])
    beta = mg * (-hg * (G + twoR) + bg * twoR + lg * G)
    d = mg * hg + beta * c1
    cvec = np.array([beta * c0, -beta * c1 + mg * (lg - hg)])

    h = np.zeros(HLEN)
    h[0] = d
    Ajb = bvec.copy()
    for m in range(1, HLEN):
        h[m] = cvec @ Ajb
        Ajb = A @ Ajb
    hmax = np.abs(h).max() + 1e-300
    assert np.abs(h[129:]).max() < 1e-9 * hmax, (
        "filter impulse response does not decay within 128 taps; "
        "FIR reformulation invalid for these parameters"
    )

    # Band matrix hband[k, i] = h[(i - 128) - k]: for an x-block at offset
    # 128*m relative to the output chunk start, the matmul rhs is
    # hband[:, 128*(1-m) : 128*(1-m) + chunk]  (m in {-1, 0, ...}).
    k = np.arange(128)[:, None]
    i = np.arange(512)[None, :]
    midx = (i - 128) - k
    hband = np.where((midx >= 0) & (midx < HLEN), h[np.clip(midx, 0, HLEN - 1)], 0.0)

    # Initial-state correction: y[:, t] += (c A^t) . v0 for t < chunk.
    cv0 = np.zeros((2, chunk))
    cAt = cvec.copy()
    for t in range(chunk):
        cv0[:, t] = cAt
        cAt = cAt @ A

    # Final state: vfin[b] = sum_k x[b, T-128+k] * (A^{127-k} b).
    kv = np.zeros((128, 2))
    Aj = np.eye(2)
    for j in range(128):
        kv[127 - j] = Aj @ bvec
        Aj = A @ Aj

    return (
        np.ascontiguousarray(hband, dtype=np.float32),
        np.ascontiguousarray(cv0, dtype=np.float32),
        np.ascontiguousarray(kv, dtype=np.float32),
    )


def _build_bass(conv_f32r):
    # f32r streams through the PE at full rate when the moving dim is >=256;
    # plain fp32 pays 4 cycles/row, so narrower chunks (less window overlap
    # waste) win there.
    chunk = 256 if conv_f32r else 128
    cdt = F32R if conv_f32r else F32
    nchunk = T // chunk
    nb = chunk // 128  # x-blocks per chunk

    nc = bacc.Bacc(None, target_bir_lowering=False, debug=False)
    x_d = nc.dram_tensor("x", [B_CORE, T], F32, kind="ExternalInput")
    v_d = nc.dram_tensor("v", [B_CORE, 2], F32, kind="ExternalInput")
    hband_d = nc.dram_tensor("hband", [128, 512], F32, kind="ExternalInput")
    cv0_d = nc.dram_tensor("cv0", [2, chunk], F32, kind="ExternalInput")
    kv_d = nc.dram_tensor("kv", [128, 2], F32, kind="ExternalInput")
    y_d = nc.dram_tensor("y", [B_CORE, T], F32, kind="ExternalOutput")
    vfin_d = nc.dram_tensor("vfin", [B_CORE, 2], F32, kind="ExternalOutput")

    # alternate PSUM->SBUF copies between DVE and ACT
    copy_engines = [nc.vector.tensor_copy, nc.scalar.copy]
    copy_idx = [0]

    def copy_alt(out, in_):
        copy_engines[copy_idx[0] % 2](out, in_)
        copy_idx[0] += 1

    with tile.TileContext(nc) as tc:
        with (
            tc.tile_pool(name="consts", bufs=1) as consts,
            tc.tile_pool(name="xs", bufs=NGROUP * (T // QCOL)) as xs_pool,
            tc.tile_pool(name="xt", bufs=4) as xt_pool,
            tc.tile_pool(name="ystg", bufs=3) as ystg_pool,
            tc.tile_pool(name="vf", bufs=2) as vf_pool,
            tc.tile_pool(name="xt_ps", bufs=3, space="PSUM") as xt_psum,
            tc.tile_pool(name="y_ps", bufs=2, space="PSUM") as y_psum,
            tc.tile_pool(name="vf_ps", bufs=1, space="PSUM") as vf_psum,
        ):
            ident = consts.tile([128, 128], F32)
            masks.make_identity(nc, ident[:])
            hband_sb = consts.tile([128, 512], F32)
            nc.sync.dma_start(hband_sb[:], hband_d[:, :])
            cv0_sb = consts.tile([2, chunk], F32)
            nc.sync.dma_start(cv0_sb[:], cv0_d[:, :])
            kv_sb = consts.tile([128, 2], F32)
            nc.sync.dma_start(kv_sb[:], kv_d[:, :])
            # v transposed on the fly by a (tiny) strided DMA: [256, 2] -> [2, 256]
            vT_sb = consts.tile([2, B_CORE], F32)
            nc.sync.dma_start(vT_sb[:], v_d[:, :].rearrange("a b -> b a"))

            if conv_f32r:
                # round the band matrix and v-final kernel to FP32R once
                hband_c = consts.tile([128, 512], F32R)
                nc.vector.tensor_copy(hband_c[:], hband_sb[:])
                kv_c = consts.tile([128, 2], F32R)
                nc.vector.tensor_copy(kv_c[:], kv_sb[:])
            else:
                hband_c = hband_sb
                kv_c = kv_sb

            # input loads: 1 MiB DMAs, all of x resident in SBUF
            xs = {}
            for g in range(NGROUP):
                for q in range(T // QCOL):
                    t_sb = xs_pool.tile([128, QCOL], F32)
                    nc.sync.dma_start(
                        t_sb[:],
                        x_d[g * 128 : (g + 1) * 128, q * QCOL : (q + 1) * QCOL],
                    )
                    xs[(g, q)] = t_sb

            nxt = XTW // 128  # transposed blocks per staging tile
            for g in range(NGROUP):
                xt_tiles = {}  # staging-tile index -> SBUF tile of nxt blocks

                def make_xt(s, g=g):
                    """Transpose blocks [s*nxt, (s+1)*nxt) into one staging tile."""
                    xt_ps = xt_psum.tile([128, XTW], F32)
                    for j in range(nxt):
                        tb = s * nxt + j
                        q, off = divmod(tb, QCOL // 128)
                        nc.tensor.transpose(
                            xt_ps[:, j * 128 : (j + 1) * 128],
                            xs[(g, q)][:, off * 128 : (off + 1) * 128],
                            ident[:],
                        )
                    xt_sb = xt_pool.tile([128, XTW], cdt)
                    copy_alt(xt_sb[:], xt_ps[:])
                    return xt_sb

                def xt_slice(tb):
                    s = tb // nxt
                    if s not in xt_tiles:
                        xt_tiles[s] = make_xt(s)
                        if s - 2 in xt_tiles:
                            del xt_tiles[s - 2]
                    j = tb % nxt
                    return xt_tiles[s][:, j * 128 : (j + 1) * 128]

                ystg = None
                y_ps = None
                ypc = OSTAGE // chunk  # chunks per output stage
                for c in range(nchunk):
                    if c % ypc == 0:
                        ystg = ystg_pool.tile([128, OSTAGE], F32)
                        y_ps = y_psum.tile([128, OSTAGE], F32)
                    oc = c % ypc
                    yslice = y_ps[:, oc * chunk : (oc + 1) * chunk]
                    # blocks m = -1 .. nb-1 relative to chunk start
                    first = True
                    for m in range(-1, nb):
                        tb = c * nb + m
                        if tb < 0:
                            continue
                        last = (m == nb - 1) and c != 0
                        nc.tensor.matmul(
                            yslice,
                            xt_slice(tb),
                            hband_c[:, 128 * (1 - m) : 128 * (1 - m) + chunk],
                            start=first,
                            stop=last,
                        )
                        first = False
                    if c == 0:
                        # initial-state correction (rank-2, fp32)
                        nc.tensor.matmul(
                            yslice,
                            vT_sb[:, g * 128 : (g + 1) * 128],
                            cv0_sb[:],
                            start=False,
                            stop=True,
                        )
                    if oc == ypc - 1:
                        copy_alt(ystg[:], y_ps[:])
                        t0 = (c + 1) * chunk - OSTAGE
                        nc.sync.dma_start(
                            y_d[g * 128 : (g + 1) * 128, t0 : t0 + OSTAGE], ystg[:]
                        )

                # final state from the last transposed block
                vf_ps = vf_psum.tile([128, 2], F32)
                nc.tensor.matmul(
                    vf_ps[:], xt_slice(NBLK - 1), kv_c[:], start=True, stop=True
                )
                vf_sb = vf_pool.tile([128, 2], F32)
                nc.vector.tensor_copy(vf_sb[:], vf_ps[:])
                nc.sync.dma_start(vfin_d[g * 128 : (g + 1) * 128, :], vf_sb[:])

    nc.compile()
    return nc


_cache = {}


def _get_nc(conv_f32r):
    if conv_f32r not in _cache:
        _cache[conv_f32r] = _build_bass(conv_f32r)
    return _cache[conv_f32r]


def kernel(x, v, G, twoR, hp_gain, bp_gain, lp_gain, master_gain,
           conv_f32r=True, trace=False):
    x = np.ascontiguousarray(np.asarray(x, dtype=np.float32))
    v = np.ascontiguousarray(np.asarray(v, dtype=np.float32))
    assert x.shape == (B, T) and v.shape == (B, 2), (x.shape, v.shape)

    chunk = 256 if conv_f32r else 128
    hband, cv0, kv = _filter_tables(
        np.asarray(G).ravel()[0],
        np.asarray(twoR).ravel()[0],
        np.asarray(hp_gain).ravel()[0],
        np.asarray(bp_gain).ravel()[0],
        np.asarray(lp_gain).ravel()[0],
        np.asarray(master_gain).ravel()[0],
        chunk,
    )

    nc = _get_nc(conv_f32r)
    in_maps = []
    for core in range(N_CORES):
        r0 = core * B_CORE
        in_maps.append(
            {
                "x": np.ascontiguousarray(x[r0 : r0 + B_CORE]),
                "v": np.ascontiguousarray(v[r0 : r0 + B_CORE]),
                "hband": hband,
                "cv0": cv0,
                "kv": kv,
            }
        )

    res = run_bass_kernel_spmd(
        nc, in_maps, core_ids=list(range(N_CORES)), trace=trace
    )
    y = np.concatenate([r["y"] for r in res.results], axis=0)
    vfin = np.concatenate([r["vfin"] for r in res.results], axis=0)
    kernel.last_results = res
    return y, vfin


kernel.last_results = None
